# revision 2
# baseline (speedup 1.0000x reference)
"""Trainium2 Bass kernel for the gene-network AE decoder (3 sparse layers).

Network (per reference):
  h1 = tanh(x @ A1 + b1)                A1: [1024, 80000], 16 nnz/col
  h2 = tanh(blockdiag4x4(W2) h1 + b2)   gene-local 4x4 mixing
  y  = blockdiag1x4(W3) h2 + b3         gene-local 4->1 reduction

Sharding: genes across the 8 cores (2500 genes -> padded to 2560 = 10240
nodes = 20 matmul tiles of 512). No inter-core communication: layer 1 only
needs the (replicated) 1024 TF features; layers 2/3 are gene-local.

Per m-tile of 512 nodes (128 genes), the per-core pipeline is:
  DMA a1 tile (fp8 e3m4, pre-scaled x8) -> PE: 8 contraction matmuls -> PSUM
  DVE: evacuate 8*z1 to bf16 -> PE: 4 transposes -> psT [m, b]
  ACT: tanh(psT/8 + b1) per ptile (bias now per-partition) -> h1T bf16
  PE: 4 block-diag W2 matmuls -> ACT: tanh(+b2) -> h2T
  PE: 4 block-diag W3 matmuls packed into one PSUM tile
  DVE: + b3 -> bf16 -> DMA out rows (m-major output, host transposes back)

The layer-1 sparse matrix is expanded to dense fp8 on the host (placement
of the runtime w1 values at positions given by the runtime in1 indices; all
arithmetic happens on device). fp8 e3m4 halves the dominant HBM stream
(21 MB -> 10.5 MB per core); w2/w3 stay bf16 to keep the accumulated
quantization error ~1.3% (gate is 2%).
"""

import sys
import types

import numpy as np

try:
    import ml_dtypes
except ImportError:  # pragma: no cover
    ml_dtypes = None

import concourse.bass as bass
import concourse.tile as tile
from concourse import bacc, mybir
from concourse.bass_utils import run_bass_kernel_spmd

# ---------------------------------------------------------------- constants
B = 128          # batch
N_TF = 1024      # input features (= 8 chunks of 128)
N_GENES = 20000
W = 4            # nodes per gene
FANIN = 16
NCORES = 8
GC = N_GENES // NCORES      # 2500 genes / core
GP = 2560                   # padded genes / core
MP = GP * W                 # 10240 padded nodes / core
MT = 512                    # matmul moving tile (1 PSUM bank of f32)
NT = MP // MT               # 20 tiles / core
NCH = N_TF // 128           # 8 contraction chunks
A1SCALE = 8.0    # fp8e3 pre-scale: keeps w1 out of the e3m4 subnormal range
NPT = MP // 128             # 80 ptiles (128 nodes = 32 genes)

BF16 = mybir.dt.bfloat16
F32 = mybir.dt.float32
FP8 = mybir.dt.float8e3

_COMPILED = None


def _np_bf16():
    assert ml_dtypes is not None, "ml_dtypes required for bf16 host arrays"
    return ml_dtypes.bfloat16


def _np_fp8():
    assert ml_dtypes is not None, "ml_dtypes required for fp8 host arrays"
    return ml_dtypes.float8_e3m4


# ---------------------------------------------------------------- NTFF shim
def _install_ntff_shim():
    """Register the NTFF profile hook if this image's antenv lacks it."""
    try:
        import antenv
        if "antenv.axon_hooks" in sys.modules:
            return
        mod = types.ModuleType("antenv.axon_hooks")
        mod._hook = None
        mod.set_axon_ntff_profile_hook = lambda h: setattr(mod, "_hook", h)
        mod.get_axon_ntff_profile_hook = lambda: mod._hook
        sys.modules["antenv.axon_hooks"] = mod
        antenv.axon_hooks = mod
        from trn_agent_boot.trn_boot import _ntff_profile_via_ctypes
        mod.set_axon_ntff_profile_hook(
            _ntff_profile_via_ctypes("/opt/axon/libaxon_pjrt.so"))
    except Exception:
        pass


# ---------------------------------------------------------------- program
def _build_program():
    nc = bacc.Bacc("TRN2", target_bir_lowering=False, debug=False,
                   num_devices=NCORES)

    a1_ext = nc.dram_tensor("a1", [NT * 128, NCH * MT], FP8,
                            kind="ExternalInput")
    xt_ext = nc.dram_tensor("xt", [128, N_TF], BF16, kind="ExternalInput")
    b1_ext = nc.dram_tensor("b1c", [128, NPT], F32, kind="ExternalInput")
    w2_ext = nc.dram_tensor("w2m", [128, NPT * 128], BF16,
                            kind="ExternalInput")
    w3_ext = nc.dram_tensor("w3m", [128, NPT * 32], BF16,
                            kind="ExternalInput")
    b2_ext = nc.dram_tensor("b2c", [128, NPT], F32, kind="ExternalInput")
    b3_ext = nc.dram_tensor("b3c", [128, NT], F32, kind="ExternalInput")
    id_ext = nc.dram_tensor("ident", [128, 128], BF16, kind="ExternalInput")
    out_ext = nc.dram_tensor("out", [MP // W, 128], BF16,
                             kind="ExternalOutput")

    with tile.TileContext(nc) as tc:
        with (
            tc.tile_pool(name="consts", bufs=1) as consts,
            tc.tile_pool(name="a1p", bufs=3) as a1p,
            tc.tile_pool(name="w2sp", bufs=4) as w2sp,
            tc.tile_pool(name="w3sp", bufs=4) as w3sp,
            tc.tile_pool(name="ps1p", bufs=3, space="PSUM") as ps1p,
            tc.tile_pool(name="psTp", bufs=2, space="PSUM") as psTp,
            tc.tile_pool(name="ps2p", bufs=2, space="PSUM") as ps2p,
            tc.tile_pool(name="ps3p", bufs=1, space="PSUM") as ps3p,
            tc.tile_pool(name="s1p", bufs=3) as s1p,
            tc.tile_pool(name="h1Tp", bufs=3) as h1Tp,
            tc.tile_pool(name="h2Tp", bufs=3) as h2Tp,
            tc.tile_pool(name="outp", bufs=3) as outp,
        ):
            xt = consts.tile([128, N_TF], BF16, tag="xt")
            nc.sync.dma_start(xt[:], xt_ext.ap())
            b1c = consts.tile([128, NPT], F32, tag="b1c")
            nc.sync.dma_start(b1c[:], b1_ext.ap())
            b2c = consts.tile([128, NPT], F32, tag="b2c")
            nc.sync.dma_start(b2c[:], b2_ext.ap())
            b3c = consts.tile([128, NT], F32, tag="b3c")
            nc.sync.dma_start(b3c[:], b3_ext.ap())
            ident = consts.tile([128, 128], BF16, tag="ident")
            nc.sync.dma_start(ident[:], id_ext.ap())

            def tail(tt, ps1, w2t, w3t):
                """evac -> transpose -> tanh+b1 -> layer 2 -> layer 3 -> out."""
                s1 = s1p.tile([128, MT], BF16, tag="s1", name=f"s1_{tt}")
                nc.vector.tensor_copy(s1[:], ps1[:])

                # ---- transpose to [m, b] via PE
                psT = psTp.tile([128, MT], BF16, tag="psT", name=f"psT_{tt}")
                for q in range(4):
                    nc.tensor.transpose(psT[:, q * 128:(q + 1) * 128],
                                        s1[:, q * 128:(q + 1) * 128],
                                        ident[:])
                # ---- tanh(z/8 + b1) with per-partition bias -> h1T [m, b]
                h1T = h1Tp.tile([128, MT], BF16, tag="h1T", name=f"h1T_{tt}")
                for q in range(4):
                    pt = tt * 4 + q
                    nc.scalar.activation(h1T[:, q * 128:(q + 1) * 128],
                                         psT[:, q * 128:(q + 1) * 128],
                                         mybir.ActivationFunctionType.Tanh,
                                         bias=b1c[:, pt:pt + 1],
                                         scale=1.0 / A1SCALE)

                # ---- layer 2: block-diag W2 matmuls per 128-node ptile
                ps2 = ps2p.tile([128, MT], F32, tag="ps2", name=f"ps2_{tt}")
                for q in range(4):
                    nc.tensor.matmul(ps2[:, q * 128:(q + 1) * 128],
                                     w2t[:, q * 128:(q + 1) * 128],
                                     h1T[:, q * 128:(q + 1) * 128],
                                     start=True, stop=True)
                h2T = h2Tp.tile([128, MT], BF16, tag="h2T", name=f"h2T_{tt}")
                for q in range(4):
                    pt = tt * 4 + q
                    nc.scalar.activation(h2T[:, q * 128:(q + 1) * 128],
                                         ps2[:, q * 128:(q + 1) * 128],
                                         mybir.ActivationFunctionType.Tanh,
                                         bias=b2c[:, pt:pt + 1])

                # ---- layer 3: 4->1 per gene, packed into one PSUM tile
                ps3 = ps3p.tile([128, 128], F32, tag="ps3", name=f"ps3_{tt}")
                for q in range(4):
                    nc.tensor.matmul(ps3[q * 32:(q + 1) * 32, :],
                                     w3t[:, q * 32:(q + 1) * 32],
                                     h2T[:, q * 128:(q + 1) * 128],
                                     start=True, stop=True,
                                     tile_position=(0, 32 * q))
                yt = outp.tile([128, 128], BF16, tag="yt", name=f"yt_{tt}")
                nc.vector.tensor_scalar_add(yt[:], ps3[:], b3c[:, tt:tt + 1])
                nc.gpsimd.dma_start(out_ext.ap()[tt * 128:(tt + 1) * 128, :],
                                    yt[:])

            # software pipeline: layer-1 matmuls for tile tt run while the
            # tail (transpose/tanh/L2/L3) of tile tt-LAG executes, so PE is
            # never waiting on a fresh tanh
            LAG = 2
            ps1s = {}
            tails = {}
            for tt in range(NT + LAG):
                if tt < NT:
                    a1t = a1p.tile([128, NCH * MT], FP8,
                                   tag="a1t", name=f"a1t_{tt}")
                    nc.sync.dma_start(a1t[:],
                                      a1_ext.ap()[tt * 128:(tt + 1) * 128, :])
                    w2t = w2sp.tile([128, MT], BF16, tag="w2t",
                                    name=f"w2t_{tt}")
                    nc.scalar.dma_start(w2t[:],
                                        w2_ext.ap()[:, tt * 512:(tt + 1) * 512])
                    w3t = w3sp.tile([128, 128], BF16, tag="w3t",
                                    name=f"w3t_{tt}")
                    nc.scalar.dma_start(w3t[:],
                                        w3_ext.ap()[:, tt * 128:(tt + 1) * 128])
                    tails[tt] = (w2t, w3t)
                    ps1 = ps1p.tile([128, MT], F32, tag="ps1",
                                    name=f"ps1_{tt}")
                    for ch in range(NCH):
                        nc.tensor.matmul(ps1[:],
                                         xt[:, ch * 128:(ch + 1) * 128],
                                         a1t[:, ch * MT:(ch + 1) * MT],
                                         start=(ch == 0), stop=(ch == NCH - 1))
                    ps1s[tt] = ps1
                if tt >= LAG:
                    w2t, w3t = tails.pop(tt - LAG)
                    tail(tt - LAG, ps1s.pop(tt - LAG), w2t, w3t)

    nc.compile()
    return nc


# ---------------------------------------------------------------- host prep
def _prep_core(c, w1, b1, w2, b2, w3, b3, in1):
    """Build the per-core input arrays (index/layout placement only)."""
    bf16 = _np_bf16()
    fp8 = _np_fp8()
    MC = GC * W  # 10000 real nodes per core

    # --- layer-1 dense matrix [1024, MP], columns = local node id 4g+j
    m_glob0 = (GC * c) * W
    e_idx = m_glob0 * FANIN + np.arange(MC * FANIN)
    t = in1[e_idx].astype(np.int64)                 # [MC*16]
    wv = w1[e_idx].astype(np.float64)
    mloc = np.repeat(np.arange(MC, dtype=np.int64), FANIN)
    A1 = np.bincount(t * MP + mloc, weights=wv,
                     minlength=N_TF * MP).reshape(N_TF, MP)
    a1_packed = ((A1 * A1SCALE).reshape(NCH, 128, NT, MT)
                 .transpose(2, 1, 0, 3)
                 .reshape(NT * 128, NCH * MT)
                 .astype(np.float32).astype(fp8))

    b1p = np.zeros(MP, np.float32)
    b1p[:MC] = b1[m_glob0:m_glob0 + MC]
    b1c = np.ascontiguousarray(b1p.reshape(NPT, 128).T)

    # --- padded per-gene weights
    w2pad = np.zeros((GP, W, W), np.float32)        # [gene, i, j]
    w2pad[:GC] = w2.reshape(N_GENES, W, W)[GC * c:GC * (c + 1)]
    b2pad = np.zeros((GP, W), np.float32)
    b2pad[:GC] = b2.reshape(N_GENES, W)[GC * c:GC * (c + 1)]
    w3pad = np.zeros((GP, W), np.float32)
    w3pad[:GC] = w3.reshape(N_GENES, W)[GC * c:GC * (c + 1)]
    b3pad = np.zeros(GP, np.float32)
    b3pad[:GC] = b3[GC * c:GC * (c + 1)]

    # --- W2 block-diag mats: W2m[pt, (a,j), (b,i)] = d(a==b) w2[g,i,j]
    idx = np.arange(32)
    W2m = np.zeros((NPT, 32, W, 32, W), np.float32)
    W2m[:, idx, :, idx, :] = (w2pad.reshape(NPT, 32, W, W)
                              .transpose(1, 0, 3, 2))  # -> [a, pt, j, i]
    w2m = (W2m.reshape(NPT, 128, 128).transpose(1, 0, 2)
           .reshape(128, NPT * 128).astype(bf16))

    # --- W3 mats: W3m[pt, (a,i), b] = d(a==b) w3[g*4+i]
    W3m = np.zeros((NPT, 32, W, 32), np.float32)
    W3m[:, idx, :, idx] = w3pad.reshape(NPT, 32, W).transpose(1, 0, 2)
    w3m = (W3m.reshape(NPT, 128, 32).transpose(1, 0, 2)
           .reshape(128, NPT * 32).astype(bf16))

    # --- bias columns
    b2c = (b2pad.reshape(NPT, 32, W).transpose(1, 2, 0)
           .reshape(128, NPT).astype(np.float32))
    b3c = np.ascontiguousarray(b3pad.reshape(NT, 128).T)

    return {
        "a1": a1_packed,
        "b1c": b1c,
        "w2m": w2m,
        "w3m": w3m,
        "b2c": b2c,
        "b3c": b3c,
    }


def _run(inputs, trace=False):
    global _COMPILED
    if _COMPILED is None:
        _COMPILED = _build_program()
    nc = _COMPILED

    bf16 = _np_bf16()
    features = np.asarray(inputs["features"], np.float32)
    w1 = np.asarray(inputs["w1"], np.float32)
    b1 = np.asarray(inputs["b1"], np.float32)
    w2 = np.asarray(inputs["w2"], np.float32)
    b2 = np.asarray(inputs["b2"], np.float32)
    w3 = np.asarray(inputs["w3"], np.float32)
    b3 = np.asarray(inputs["b3"], np.float32)
    in1 = np.asarray(inputs["in1"], np.int32)

    # stationary x: [p, ch*128 + b] = x[b, ch*128 + p]
    xt = (features.T.reshape(NCH, 128, B).transpose(1, 0, 2)
          .reshape(128, N_TF).astype(bf16))
    ident = np.eye(128, dtype=np.float32).astype(bf16)

    in_maps = []
    for c in range(NCORES):
        m = _prep_core(c, w1, b1, w2, b2, w3, b3, in1)
        m["xt"] = xt
        m["ident"] = ident
        in_maps.append(m)

    if trace:
        _install_ntff_shim()
    res = run_bass_kernel_spmd(nc, in_maps, core_ids=list(range(NCORES)),
                               trace=trace)
    y = np.empty((B, N_GENES), np.float32)
    for c in range(NCORES):
        yc = np.asarray(res.results[c]["out"]).astype(np.float32)  # [2560,128]
        y[:, GC * c:GC * (c + 1)] = yc[:GC, :].T
    return y, res.exec_time_ns


def kernel(**inputs) -> np.ndarray:
    y, _ = _run(inputs, trace=False)
    return y


# revision 8
# speedup vs baseline: 1.1901x; 1.1901x over previous
"""Trainium2 Bass kernel for the gene-network AE decoder (3 sparse layers).

Network (per reference):
  h1 = tanh(x @ A1 + b1)                A1: [1024, 80000], 16 nnz/col
  h2 = tanh(blockdiag4x4(W2) h1 + b2)   gene-local 4x4 mixing
  y  = blockdiag1x4(W3) h2 + b3         gene-local 4->1 reduction

Sharding: genes across the 8 cores (2500 genes -> padded to 2560 = 10240
nodes = 20 matmul tiles of 512). No inter-core communication: layer 1 only
needs the (replicated) 1024 TF features; layers 2/3 are gene-local.

Per m-tile of 512 nodes (128 genes), the per-core pipeline is:
  DMA a1 tile (fp8 e3m4, pre-scaled x8) -> PE: 8 contraction matmuls -> PSUM
  DVE: evacuate 8*z1 to bf16 -> PE: 4 transposes -> psT [m, b]
  ACT: tanh(psT/8 + b1) per ptile (bias now per-partition) -> h1T bf16
  PE: 4 block-diag W2 matmuls -> ACT: tanh(+b2) -> h2T
  PE: 4 block-diag W3 matmuls packed into one PSUM tile
  DVE: + b3 -> bf16 -> DMA out rows (m-major output, host transposes back)

The layer-1 sparse matrix is expanded to dense fp8 on the host (placement
of the runtime w1 values at positions given by the runtime in1 indices; all
arithmetic happens on device). fp8 e3m4 halves the dominant HBM stream
(21 MB -> 10.5 MB per core); w2/w3 stay bf16 to keep the accumulated
quantization error ~1.3% (gate is 2%).
"""

import sys
import types

import numpy as np

try:
    import ml_dtypes
except ImportError:  # pragma: no cover
    ml_dtypes = None

import concourse.bass as bass
import concourse.tile as tile
from concourse import bacc, mybir
from concourse.bass_utils import run_bass_kernel_spmd

# ---------------------------------------------------------------- constants
B = 128          # batch
N_TF = 1024      # input features (= 8 chunks of 128)
N_GENES = 20000
W = 4            # nodes per gene
FANIN = 16
NCORES = 8
GC = N_GENES // NCORES      # 2500 genes / core
GP = 2560                   # padded genes / core
MP = GP * W                 # 10240 padded nodes / core
MT = 512                    # matmul moving tile (1 PSUM bank of f32)
NT = MP // MT               # 20 tiles / core
NCH = N_TF // 128           # 8 contraction chunks
A1SCALE = 8.0    # fp8e3 pre-scale: keeps w1 out of the e3m4 subnormal range
NPT = MP // 128             # 80 ptiles (128 nodes = 32 genes)

BF16 = mybir.dt.bfloat16
F32 = mybir.dt.float32
FP8 = mybir.dt.float8e3

_COMPILED = None


def _np_bf16():
    assert ml_dtypes is not None, "ml_dtypes required for bf16 host arrays"
    return ml_dtypes.bfloat16


def _np_fp8():
    assert ml_dtypes is not None, "ml_dtypes required for fp8 host arrays"
    return ml_dtypes.float8_e3m4


# ---------------------------------------------------------------- NTFF shim
def _install_ntff_shim():
    """Register the NTFF profile hook if this image's antenv lacks it."""
    try:
        import antenv
        if "antenv.axon_hooks" in sys.modules:
            return
        mod = types.ModuleType("antenv.axon_hooks")
        mod._hook = None
        mod.set_axon_ntff_profile_hook = lambda h: setattr(mod, "_hook", h)
        mod.get_axon_ntff_profile_hook = lambda: mod._hook
        sys.modules["antenv.axon_hooks"] = mod
        antenv.axon_hooks = mod
        from trn_agent_boot.trn_boot import _ntff_profile_via_ctypes
        mod.set_axon_ntff_profile_hook(
            _ntff_profile_via_ctypes("/opt/axon/libaxon_pjrt.so"))
    except Exception:
        pass


# ---------------------------------------------------------------- program
def _build_program():
    nc = bacc.Bacc("TRN2", target_bir_lowering=False, debug=False,
                   num_devices=NCORES)

    a1_ext = nc.dram_tensor("a1", [NT * 128, NCH * MT], FP8,
                            kind="ExternalInput")
    xt_ext = nc.dram_tensor("xt", [128, N_TF], BF16, kind="ExternalInput")
    b1_ext = nc.dram_tensor("b1c", [128, NPT], BF16, kind="ExternalInput")
    w2_ext = nc.dram_tensor("w2m", [128, NPT * 128], BF16,
                            kind="ExternalInput")
    w3_ext = nc.dram_tensor("w3m", [128, NPT * 32], BF16,
                            kind="ExternalInput")
    b2_ext = nc.dram_tensor("b2c", [128, NPT], F32, kind="ExternalInput")
    b3_ext = nc.dram_tensor("b3c", [128, NT], F32, kind="ExternalInput")
    id_ext = nc.dram_tensor("ident", [128, 128], BF16, kind="ExternalInput")
    out_ext = nc.dram_tensor("out", [MP // W, 128], BF16,
                             kind="ExternalOutput")

    with tile.TileContext(nc) as tc:
        with (
            tc.tile_pool(name="consts", bufs=1) as consts,
            tc.tile_pool(name="a1p", bufs=3) as a1p,
            tc.tile_pool(name="w2sp", bufs=4) as w2sp,
            tc.tile_pool(name="w3sp", bufs=4) as w3sp,
            tc.tile_pool(name="ps1p", bufs=3, space="PSUM") as ps1p,
            tc.tile_pool(name="psTp", bufs=2, space="PSUM") as psTp,
            tc.tile_pool(name="ps2p", bufs=2, space="PSUM") as ps2p,
            tc.tile_pool(name="ps3p", bufs=1, space="PSUM") as ps3p,
            tc.tile_pool(name="s1p", bufs=3) as s1p,
            tc.tile_pool(name="s2p", bufs=3) as s2p,
            tc.tile_pool(name="s3p", bufs=3) as s3p,
            tc.tile_pool(name="h1Tp", bufs=3) as h1Tp,
            tc.tile_pool(name="h2Tp", bufs=3) as h2Tp,
            tc.tile_pool(name="outp", bufs=3) as outp,
        ):
            xt = consts.tile([128, N_TF], BF16, tag="xt")
            nc.sync.dma_start(xt[:], xt_ext.ap())
            b1c = consts.tile([128, NPT], BF16, tag="b1c")
            nc.sync.dma_start(b1c[:], b1_ext.ap())
            b2c = consts.tile([128, NPT], F32, tag="b2c")
            nc.sync.dma_start(b2c[:], b2_ext.ap())
            b3c = consts.tile([128, NT], F32, tag="b3c")
            nc.sync.dma_start(b3c[:], b3_ext.ap())
            ident = consts.tile([128, 128], BF16, tag="ident")
            nc.sync.dma_start(ident[:], id_ext.ap())

            def tail(tt, ps1, w2t, w3t):
                """evac -> transpose -> +b1,tanh -> layer 2 -> layer 3 -> out."""
                # ---- evacuate z1*8 from PSUM, applying the 1/8 unscale
                s1 = s1p.tile([128, MT], BF16, tag="s1", name=f"s1_{tt}")
                nc.scalar.activation(s1[:], ps1[:],
                                     mybir.ActivationFunctionType.Copy,
                                     scale=1.0 / A1SCALE)

                # ---- transpose to [m, b] via PE
                psT = psTp.tile([128, MT], BF16, tag="psT", name=f"psT_{tt}")
                for q in range(4):
                    nc.tensor.transpose(psT[:, q * 128:(q + 1) * 128],
                                        s1[:, q * 128:(q + 1) * 128],
                                        ident[:])
                # ---- z1 + b1 (b1 broadcast over batch), then tanh -> h1T
                s2 = s2p.tile([128, MT], BF16, tag="s2", name=f"s2_{tt}")
                nc.vector.tensor_tensor(
                    s2[:].rearrange("p (q b) -> p q b", q=4),
                    psT[:].rearrange("p (q b) -> p q b", q=4),
                    b1c[:, tt * 4:(tt + 1) * 4, None].to_broadcast(
                        [128, 4, 128]),
                    mybir.AluOpType.add)
                h1T = h1Tp.tile([128, MT], BF16, tag="h1T", name=f"h1T_{tt}")
                nc.scalar.activation(h1T[:], s2[:],
                                     mybir.ActivationFunctionType.Tanh)

                # ---- layer 2: block-diag W2 matmuls per 128-node ptile
                ps2 = ps2p.tile([128, MT], F32, tag="ps2", name=f"ps2_{tt}")
                for q in range(4):
                    nc.tensor.matmul(ps2[:, q * 128:(q + 1) * 128],
                                     w2t[:, q * 128:(q + 1) * 128],
                                     h1T[:, q * 128:(q + 1) * 128],
                                     start=True, stop=True)
                s3 = s3p.tile([128, MT], F32, tag="s3", name=f"s3_{tt}")
                nc.vector.tensor_tensor(
                    s3[:].rearrange("p (q b) -> p q b", q=4),
                    ps2[:].rearrange("p (q b) -> p q b", q=4),
                    b2c[:, tt * 4:(tt + 1) * 4, None].to_broadcast(
                        [128, 4, 128]),
                    mybir.AluOpType.add)
                h2T = h2Tp.tile([128, MT], BF16, tag="h2T", name=f"h2T_{tt}")
                nc.scalar.activation(h2T[:], s3[:],
                                     mybir.ActivationFunctionType.Tanh)

                # ---- layer 3: 4->1 per gene, packed into one PSUM tile
                ps3 = ps3p.tile([128, 128], F32, tag="ps3", name=f"ps3_{tt}")
                for q in range(4):
                    nc.tensor.matmul(ps3[q * 32:(q + 1) * 32, :],
                                     w3t[:, q * 32:(q + 1) * 32],
                                     h2T[:, q * 128:(q + 1) * 128],
                                     start=True, stop=True,
                                     tile_position=(0, 32 * q))
                yt = outp.tile([128, 128], BF16, tag="yt", name=f"yt_{tt}")
                nc.vector.tensor_scalar_add(yt[:], ps3[:], b3c[:, tt:tt + 1])
                nc.sync.dma_start(out_ext.ap()[tt * 128:(tt + 1) * 128, :],
                                  yt[:])

            # software pipeline: layer-1 matmuls for tile tt run while the
            # tail (transpose/tanh/L2/L3) of tile tt-LAG executes, so PE is
            # never waiting on a fresh tanh
            LAG = 2
            ps1s = {}
            tails = {}
            for tt in range(NT + LAG):
                if tt < NT:
                    a1t = a1p.tile([128, NCH * MT], FP8,
                                   tag="a1t", name=f"a1t_{tt}")
                    nc.sync.dma_start(a1t[:],
                                      a1_ext.ap()[tt * 128:(tt + 1) * 128, :])
                    w2t = w2sp.tile([128, MT], BF16, tag="w2t",
                                    name=f"w2t_{tt}")
                    nc.scalar.dma_start(w2t[:],
                                        w2_ext.ap()[:, tt * 512:(tt + 1) * 512])
                    w3t = w3sp.tile([128, 128], BF16, tag="w3t",
                                    name=f"w3t_{tt}")
                    nc.scalar.dma_start(w3t[:],
                                        w3_ext.ap()[:, tt * 128:(tt + 1) * 128])
                    tails[tt] = (w2t, w3t)
                    ps1 = ps1p.tile([128, MT], F32, tag="ps1",
                                    name=f"ps1_{tt}")
                    for ch in range(NCH):
                        nc.tensor.matmul(ps1[:],
                                         xt[:, ch * 128:(ch + 1) * 128],
                                         a1t[:, ch * MT:(ch + 1) * MT],
                                         start=(ch == 0), stop=(ch == NCH - 1))
                    ps1s[tt] = ps1
                if tt >= LAG:
                    w2t, w3t = tails.pop(tt - LAG)
                    tail(tt - LAG, ps1s.pop(tt - LAG), w2t, w3t)

    nc.compile()
    return nc


# ---------------------------------------------------------------- host prep
def _prep_core(c, w1, b1, w2, b2, w3, b3, in1):
    """Build the per-core input arrays (index/layout placement only)."""
    bf16 = _np_bf16()
    fp8 = _np_fp8()
    MC = GC * W  # 10000 real nodes per core

    # --- layer-1 dense matrix [1024, MP], columns = local node id 4g+j
    m_glob0 = (GC * c) * W
    e_idx = m_glob0 * FANIN + np.arange(MC * FANIN)
    t = in1[e_idx].astype(np.int64)                 # [MC*16]
    wv = w1[e_idx].astype(np.float64)
    mloc = np.repeat(np.arange(MC, dtype=np.int64), FANIN)
    A1 = np.bincount(t * MP + mloc, weights=wv,
                     minlength=N_TF * MP).reshape(N_TF, MP)
    a1_packed = ((A1 * A1SCALE).reshape(NCH, 128, NT, MT)
                 .transpose(2, 1, 0, 3)
                 .reshape(NT * 128, NCH * MT)
                 .astype(np.float32).astype(fp8))

    b1p = np.zeros(MP, np.float32)
    b1p[:MC] = b1[m_glob0:m_glob0 + MC]
    b1c = np.ascontiguousarray(b1p.reshape(NPT, 128).T).astype(bf16)

    # --- padded per-gene weights
    w2pad = np.zeros((GP, W, W), np.float32)        # [gene, i, j]
    w2pad[:GC] = w2.reshape(N_GENES, W, W)[GC * c:GC * (c + 1)]
    b2pad = np.zeros((GP, W), np.float32)
    b2pad[:GC] = b2.reshape(N_GENES, W)[GC * c:GC * (c + 1)]
    w3pad = np.zeros((GP, W), np.float32)
    w3pad[:GC] = w3.reshape(N_GENES, W)[GC * c:GC * (c + 1)]
    b3pad = np.zeros(GP, np.float32)
    b3pad[:GC] = b3[GC * c:GC * (c + 1)]

    # --- W2 block-diag mats: W2m[pt, (a,j), (b,i)] = d(a==b) w2[g,i,j]
    idx = np.arange(32)
    W2m = np.zeros((NPT, 32, W, 32, W), np.float32)
    W2m[:, idx, :, idx, :] = (w2pad.reshape(NPT, 32, W, W)
                              .transpose(1, 0, 3, 2))  # -> [a, pt, j, i]
    w2m = (W2m.reshape(NPT, 128, 128).transpose(1, 0, 2)
           .reshape(128, NPT * 128).astype(bf16))

    # --- W3 mats: W3m[pt, (a,i), b] = d(a==b) w3[g*4+i]
    W3m = np.zeros((NPT, 32, W, 32), np.float32)
    W3m[:, idx, :, idx] = w3pad.reshape(NPT, 32, W).transpose(1, 0, 2)
    w3m = (W3m.reshape(NPT, 128, 32).transpose(1, 0, 2)
           .reshape(128, NPT * 32).astype(bf16))

    # --- bias columns
    b2c = (b2pad.reshape(NPT, 32, W).transpose(1, 2, 0)
           .reshape(128, NPT).astype(np.float32))
    b3c = np.ascontiguousarray(b3pad.reshape(NT, 128).T)

    return {
        "a1": a1_packed,
        "b1c": b1c,
        "w2m": w2m,
        "w3m": w3m,
        "b2c": b2c,
        "b3c": b3c,
    }


def _run(inputs, trace=False):
    global _COMPILED
    if _COMPILED is None:
        _COMPILED = _build_program()
    nc = _COMPILED

    bf16 = _np_bf16()
    features = np.asarray(inputs["features"], np.float32)
    w1 = np.asarray(inputs["w1"], np.float32)
    b1 = np.asarray(inputs["b1"], np.float32)
    w2 = np.asarray(inputs["w2"], np.float32)
    b2 = np.asarray(inputs["b2"], np.float32)
    w3 = np.asarray(inputs["w3"], np.float32)
    b3 = np.asarray(inputs["b3"], np.float32)
    in1 = np.asarray(inputs["in1"], np.int32)

    # stationary x: [p, ch*128 + b] = x[b, ch*128 + p]
    xt = (features.T.reshape(NCH, 128, B).transpose(1, 0, 2)
          .reshape(128, N_TF).astype(bf16))
    ident = np.eye(128, dtype=np.float32).astype(bf16)

    in_maps = []
    for c in range(NCORES):
        m = _prep_core(c, w1, b1, w2, b2, w3, b3, in1)
        m["xt"] = xt
        m["ident"] = ident
        in_maps.append(m)

    if trace:
        _install_ntff_shim()
    res = run_bass_kernel_spmd(nc, in_maps, core_ids=list(range(NCORES)),
                               trace=trace)
    y = np.empty((B, N_GENES), np.float32)
    for c in range(NCORES):
        yc = np.asarray(res.results[c]["out"]).astype(np.float32)  # [2560,128]
        y[:, GC * c:GC * (c + 1)] = yc[:GC, :].T
    return y, res.exec_time_ns


def kernel(**inputs) -> np.ndarray:
    y, _ = _run(inputs, trace=False)
    return y


# revision 15
# speedup vs baseline: 1.2923x; 1.0859x over previous
"""Trainium2 Bass kernel for the gene-network AE decoder (3 sparse layers).

Network (per reference):
  h1 = tanh(x @ A1 + b1)                A1: [1024, 80000], 16 nnz/col
  h2 = tanh(blockdiag4x4(W2) h1 + b2)   gene-local 4x4 mixing
  y  = blockdiag1x4(W3) h2 + b3         gene-local 4->1 reduction

Sharding: genes across the 8 cores (2500 genes -> padded to 2560 = 10240
nodes = 20 matmul tiles of 512). No inter-core communication: layer 1 only
needs the (replicated) 1024 TF features; layers 2/3 are gene-local.

Per m-tile of 512 nodes (128 genes), the per-core pipeline is:
  DMA a1 tile (fp8 e3m4, pre-scaled x8) -> PE: 8 contraction matmuls -> PSUM
  DVE: evacuate 8*z1 to bf16 -> PE: 4 transposes -> psT [m, b]
  ACT: tanh(psT/8 + b1) per ptile (bias now per-partition) -> h1T bf16
  PE: 4 block-diag W2 matmuls -> ACT: tanh(+b2) -> h2T
  PE: 4 block-diag W3 matmuls packed into one PSUM tile
  DVE: + b3 -> bf16 -> DMA out rows (m-major output, host transposes back)

The layer-1 sparse matrix is expanded to dense fp8 on the host (placement
of the runtime w1 values at positions given by the runtime in1 indices; all
arithmetic happens on device). fp8 e3m4 halves the dominant HBM stream
(21 MB -> 10.5 MB per core); w2/w3 stay bf16 to keep the accumulated
quantization error ~1.3% (gate is 2%).
"""

import sys
import types

import numpy as np

try:
    import ml_dtypes
except ImportError:  # pragma: no cover
    ml_dtypes = None

import concourse.bass as bass
import concourse.tile as tile
from concourse import bacc, mybir
from concourse.bass_utils import run_bass_kernel_spmd

# ---------------------------------------------------------------- constants
B = 128          # batch
N_TF = 1024      # input features (= 8 chunks of 128)
N_GENES = 20000
W = 4            # nodes per gene
FANIN = 16
NCORES = 8
GC = N_GENES // NCORES      # 2500 genes / core
GP = 2560                   # padded genes / core
MP = GP * W                 # 10240 padded nodes / core
MT = 512                    # matmul moving tile (1 PSUM bank of f32)
NT = MP // MT               # 20 tiles / core
NCH = N_TF // 128           # 8 contraction chunks
A1SCALE = 8.0    # fp8e3 pre-scale: keeps w1 out of the e3m4 subnormal range
NPT = MP // 128             # 80 ptiles (128 nodes = 32 genes)

BF16 = mybir.dt.bfloat16
F32 = mybir.dt.float32
FP8 = mybir.dt.float8e3

_COMPILED = None


def _np_bf16():
    assert ml_dtypes is not None, "ml_dtypes required for bf16 host arrays"
    return ml_dtypes.bfloat16


def _np_fp8():
    assert ml_dtypes is not None, "ml_dtypes required for fp8 host arrays"
    return ml_dtypes.float8_e3m4


# ---------------------------------------------------------------- NTFF shim
def _install_ntff_shim():
    """Register the NTFF profile hook if this image's antenv lacks it."""
    try:
        import antenv
        if "antenv.axon_hooks" in sys.modules:
            return
        mod = types.ModuleType("antenv.axon_hooks")
        mod._hook = None
        mod.set_axon_ntff_profile_hook = lambda h: setattr(mod, "_hook", h)
        mod.get_axon_ntff_profile_hook = lambda: mod._hook
        sys.modules["antenv.axon_hooks"] = mod
        antenv.axon_hooks = mod
        from trn_agent_boot.trn_boot import _ntff_profile_via_ctypes
        mod.set_axon_ntff_profile_hook(
            _ntff_profile_via_ctypes("/opt/axon/libaxon_pjrt.so"))
    except Exception:
        pass


# ---------------------------------------------------------------- program
def _build_program():
    nc = bacc.Bacc("TRN2", target_bir_lowering=False, debug=False,
                   num_devices=NCORES)

    a1_ext = nc.dram_tensor("a1", [NT * 128, NCH * MT], FP8,
                            kind="ExternalInput")
    xt_ext = nc.dram_tensor("xt", [128, N_TF], BF16, kind="ExternalInput")
    b1_ext = nc.dram_tensor("b1c", [128, NPT], BF16, kind="ExternalInput")
    w2_ext = nc.dram_tensor("w2m", [128, NPT * 128], BF16,
                            kind="ExternalInput")
    w3_ext = nc.dram_tensor("w3m", [128, NPT * 32], BF16,
                            kind="ExternalInput")
    b2_ext = nc.dram_tensor("b2c", [128, NPT], F32, kind="ExternalInput")
    b3_ext = nc.dram_tensor("b3c", [128, NT], F32, kind="ExternalInput")
    id_ext = nc.dram_tensor("ident", [128, 128], BF16, kind="ExternalInput")
    out_ext = nc.dram_tensor("out", [MP // W, 128], BF16,
                             kind="ExternalOutput")

    with tile.TileContext(nc) as tc:
        with (
            tc.tile_pool(name="consts", bufs=1) as consts,
            tc.tile_pool(name="a1p", bufs=4) as a1p,
            tc.tile_pool(name="w2sp", bufs=4) as w2sp,
            tc.tile_pool(name="w3sp", bufs=4) as w3sp,
            tc.tile_pool(name="ps1p", bufs=3, space="PSUM") as ps1p,
            tc.tile_pool(name="psTp", bufs=2, space="PSUM") as psTp,
            tc.tile_pool(name="ps2p", bufs=2, space="PSUM") as ps2p,
            tc.tile_pool(name="ps3p", bufs=1, space="PSUM") as ps3p,
            tc.tile_pool(name="s1p", bufs=3) as s1p,
            tc.tile_pool(name="s2p", bufs=3) as s2p,
            tc.tile_pool(name="s3p", bufs=3) as s3p,
            tc.tile_pool(name="h1Tp", bufs=3) as h1Tp,
            tc.tile_pool(name="h2Tp", bufs=3) as h2Tp,
            tc.tile_pool(name="outp", bufs=3) as outp,
        ):
            # xt on the sync queue (first, it gates the first matmul); all
            # other consts on the scalar queue so tile 0's a1 isn't delayed
            xt = consts.tile([128, N_TF], BF16, tag="xt")
            nc.sync.dma_start(xt[:], xt_ext.ap())
            b1c = consts.tile([128, NPT], BF16, tag="b1c")
            nc.scalar.dma_start(b1c[:], b1_ext.ap())
            b2c = consts.tile([128, NPT], F32, tag="b2c")
            nc.scalar.dma_start(b2c[:], b2_ext.ap())
            b3c = consts.tile([128, NT], F32, tag="b3c")
            nc.scalar.dma_start(b3c[:], b3_ext.ap())
            ident = consts.tile([128, 128], BF16, tag="ident")
            nc.scalar.dma_start(ident[:], id_ext.ap())

            def tail(tt, ps1, w2t, w3t):
                """evac -> transpose -> +b1,tanh -> layer 2 -> layer 3 -> out."""
                # ---- evacuate z1*8 from PSUM, applying the 1/8 unscale
                s1 = s1p.tile([128, MT], BF16, tag="s1", name=f"s1_{tt}")
                nc.scalar.activation(s1[:], ps1[:],
                                     mybir.ActivationFunctionType.Copy,
                                     scale=1.0 / A1SCALE)

                # ---- transpose to [m, b] via PE
                psT = psTp.tile([128, MT], BF16, tag="psT", name=f"psT_{tt}")
                for q in range(4):
                    nc.tensor.transpose(psT[:, q * 128:(q + 1) * 128],
                                        s1[:, q * 128:(q + 1) * 128],
                                        ident[:])
                # ---- z1 + b1 (b1 broadcast over batch), then tanh -> h1T
                s2 = s2p.tile([128, MT], BF16, tag="s2", name=f"s2_{tt}")
                nc.vector.tensor_tensor(
                    s2[:].rearrange("p (q b) -> p q b", q=4),
                    psT[:].rearrange("p (q b) -> p q b", q=4),
                    b1c[:, tt * 4:(tt + 1) * 4, None].to_broadcast(
                        [128, 4, 128]),
                    mybir.AluOpType.add)
                h1T = h1Tp.tile([128, MT], BF16, tag="h1T", name=f"h1T_{tt}")
                nc.scalar.activation(h1T[:], s2[:],
                                     mybir.ActivationFunctionType.Tanh)

                # ---- layer 2: block-diag W2 matmuls per 128-node ptile
                ps2 = ps2p.tile([128, MT], F32, tag="ps2", name=f"ps2_{tt}")
                for q in range(4):
                    nc.tensor.matmul(ps2[:, q * 128:(q + 1) * 128],
                                     w2t[:, q * 128:(q + 1) * 128],
                                     h1T[:, q * 128:(q + 1) * 128],
                                     start=True, stop=True)
                s3 = s3p.tile([128, MT], F32, tag="s3", name=f"s3_{tt}")
                nc.vector.tensor_tensor(
                    s3[:].rearrange("p (q b) -> p q b", q=4),
                    ps2[:].rearrange("p (q b) -> p q b", q=4),
                    b2c[:, tt * 4:(tt + 1) * 4, None].to_broadcast(
                        [128, 4, 128]),
                    mybir.AluOpType.add)
                h2T = h2Tp.tile([128, MT], BF16, tag="h2T", name=f"h2T_{tt}")
                nc.scalar.activation(h2T[:], s3[:],
                                     mybir.ActivationFunctionType.Tanh)

                # ---- layer 3: 4->1 per gene, packed into one PSUM tile
                ps3 = ps3p.tile([128, 128], F32, tag="ps3", name=f"ps3_{tt}")
                for q in range(4):
                    nc.tensor.matmul(ps3[q * 32:(q + 1) * 32, :],
                                     w3t[:, q * 32:(q + 1) * 32],
                                     h2T[:, q * 128:(q + 1) * 128],
                                     start=True, stop=True,
                                     tile_position=(0, 32 * q))
                yt = outp.tile([128, 128], BF16, tag="yt", name=f"yt_{tt}")
                nc.vector.tensor_scalar_add(yt[:], ps3[:], b3c[:, tt:tt + 1])
                nc.sync.dma_start(out_ext.ap()[tt * 128:(tt + 1) * 128, :],
                                  yt[:])

            # software pipeline: layer-1 matmuls for tile tt run while the
            # tail (transpose/tanh/L2/L3) of tile tt-LAG executes, so PE is
            # never waiting on a fresh tanh
            LAG = 1
            ps1s = {}
            tails = {}
            for tt in range(NT + LAG):
                if tt < NT:
                    a1t = a1p.tile([128, NCH * MT], FP8,
                                   tag="a1t", name=f"a1t_{tt}")
                    nc.sync.dma_start(a1t[:],
                                      a1_ext.ap()[tt * 128:(tt + 1) * 128, :])
                    w2t = w2sp.tile([128, MT], BF16, tag="w2t",
                                    name=f"w2t_{tt}")
                    nc.scalar.dma_start(w2t[:],
                                        w2_ext.ap()[:, tt * 512:(tt + 1) * 512])
                    w3t = w3sp.tile([128, 128], BF16, tag="w3t",
                                    name=f"w3t_{tt}")
                    nc.scalar.dma_start(w3t[:],
                                        w3_ext.ap()[:, tt * 128:(tt + 1) * 128])
                    tails[tt] = (w2t, w3t)
                    ps1 = ps1p.tile([128, MT], F32, tag="ps1",
                                    name=f"ps1_{tt}")
                    for ch in range(NCH):
                        nc.tensor.matmul(ps1[:],
                                         xt[:, ch * 128:(ch + 1) * 128],
                                         a1t[:, ch * MT:(ch + 1) * MT],
                                         start=(ch == 0), stop=(ch == NCH - 1))
                    ps1s[tt] = ps1
                if tt >= LAG:
                    w2t, w3t = tails.pop(tt - LAG)
                    tail(tt - LAG, ps1s.pop(tt - LAG), w2t, w3t)

    nc.compile()
    return nc


# ---------------------------------------------------------------- host prep
def _prep_core(c, w1, b1, w2, b2, w3, b3, in1):
    """Build the per-core input arrays (index/layout placement only)."""
    bf16 = _np_bf16()
    fp8 = _np_fp8()
    MC = GC * W  # 10000 real nodes per core

    # --- layer-1 dense matrix [1024, MP], columns = local node id 4g+j
    m_glob0 = (GC * c) * W
    e_idx = m_glob0 * FANIN + np.arange(MC * FANIN)
    t = in1[e_idx].astype(np.int64)                 # [MC*16]
    wv = w1[e_idx].astype(np.float64)
    mloc = np.repeat(np.arange(MC, dtype=np.int64), FANIN)
    A1 = np.bincount(t * MP + mloc, weights=wv,
                     minlength=N_TF * MP).reshape(N_TF, MP)
    a1_packed = ((A1 * A1SCALE).reshape(NCH, 128, NT, MT)
                 .transpose(2, 1, 0, 3)
                 .reshape(NT * 128, NCH * MT)
                 .astype(np.float32).astype(fp8))

    b1p = np.zeros(MP, np.float32)
    b1p[:MC] = b1[m_glob0:m_glob0 + MC]
    b1c = np.ascontiguousarray(b1p.reshape(NPT, 128).T).astype(bf16)

    # --- padded per-gene weights
    w2pad = np.zeros((GP, W, W), np.float32)        # [gene, i, j]
    w2pad[:GC] = w2.reshape(N_GENES, W, W)[GC * c:GC * (c + 1)]
    b2pad = np.zeros((GP, W), np.float32)
    b2pad[:GC] = b2.reshape(N_GENES, W)[GC * c:GC * (c + 1)]
    w3pad = np.zeros((GP, W), np.float32)
    w3pad[:GC] = w3.reshape(N_GENES, W)[GC * c:GC * (c + 1)]
    b3pad = np.zeros(GP, np.float32)
    b3pad[:GC] = b3[GC * c:GC * (c + 1)]

    # --- W2 block-diag mats: W2m[pt, (a,j), (b,i)] = d(a==b) w2[g,i,j]
    idx = np.arange(32)
    W2m = np.zeros((NPT, 32, W, 32, W), np.float32)
    W2m[:, idx, :, idx, :] = (w2pad.reshape(NPT, 32, W, W)
                              .transpose(1, 0, 3, 2))  # -> [a, pt, j, i]
    w2m = (W2m.reshape(NPT, 128, 128).transpose(1, 0, 2)
           .reshape(128, NPT * 128).astype(bf16))

    # --- W3 mats: W3m[pt, (a,i), b] = d(a==b) w3[g*4+i]
    W3m = np.zeros((NPT, 32, W, 32), np.float32)
    W3m[:, idx, :, idx] = w3pad.reshape(NPT, 32, W).transpose(1, 0, 2)
    w3m = (W3m.reshape(NPT, 128, 32).transpose(1, 0, 2)
           .reshape(128, NPT * 32).astype(bf16))

    # --- bias columns
    b2c = (b2pad.reshape(NPT, 32, W).transpose(1, 2, 0)
           .reshape(128, NPT).astype(np.float32))
    b3c = np.ascontiguousarray(b3pad.reshape(NT, 128).T)

    return {
        "a1": a1_packed,
        "b1c": b1c,
        "w2m": w2m,
        "w3m": w3m,
        "b2c": b2c,
        "b3c": b3c,
    }


def _run(inputs, trace=False):
    global _COMPILED
    if _COMPILED is None:
        _COMPILED = _build_program()
    nc = _COMPILED

    bf16 = _np_bf16()
    features = np.asarray(inputs["features"], np.float32)
    w1 = np.asarray(inputs["w1"], np.float32)
    b1 = np.asarray(inputs["b1"], np.float32)
    w2 = np.asarray(inputs["w2"], np.float32)
    b2 = np.asarray(inputs["b2"], np.float32)
    w3 = np.asarray(inputs["w3"], np.float32)
    b3 = np.asarray(inputs["b3"], np.float32)
    in1 = np.asarray(inputs["in1"], np.int32)

    # stationary x: [p, ch*128 + b] = x[b, ch*128 + p]
    xt = (features.T.reshape(NCH, 128, B).transpose(1, 0, 2)
          .reshape(128, N_TF).astype(bf16))
    ident = np.eye(128, dtype=np.float32).astype(bf16)

    in_maps = []
    for c in range(NCORES):
        m = _prep_core(c, w1, b1, w2, b2, w3, b3, in1)
        m["xt"] = xt
        m["ident"] = ident
        in_maps.append(m)

    if trace:
        _install_ntff_shim()
    res = run_bass_kernel_spmd(nc, in_maps, core_ids=list(range(NCORES)),
                               trace=trace)
    y = np.empty((B, N_GENES), np.float32)
    for c in range(NCORES):
        yc = np.asarray(res.results[c]["out"]).astype(np.float32)  # [2560,128]
        y[:, GC * c:GC * (c + 1)] = yc[:GC, :].T
    return y, res.exec_time_ns


def kernel(**inputs) -> np.ndarray:
    y, _ = _run(inputs, trace=False)
    return y
